# revision 4
# baseline (speedup 1.0000x reference)
"""Blockwise reconditioner (block-16 normalization) on 8 Trainium2 cores.

Math per row r, block g (block size 16):
    mean = mean(x[r, 16g:16g+16])
    var  = sum((x - mean)^2) / 15          (unbiased, ddof=1)
    out  = (x - mean) / sqrt(var + 1e-5) * scales[g] + shifts[g]

Implemented as out = x * a + b with per-block coefficients
    a = scales[g] / sqrt(var + eps)
    b = shifts[g] - mean * a
using raw = sum(x^2) - sum(x)^2/16, var = raw/15.

v2: full bf16 datapath (rel-err budget is 2e-2; bf16 costs ~0.5%).
Sharding: data-parallel over rows; each of 8 cores handles [512, 8192].
Per chunk [128 rows, 2048 cols]:
  - DMA in fp32
  - ACT: convert to bf16 (xb)
  - PE: transpose xb 128x128 sub-blocks -> PSUM (bf16 transposes)
  - ACT: copy + square PSUM->SBUF into interleaved xs = [.. xT_k | sqT_k ..]
  - PE: 16 accumulating bf16 matmuls with a 0/1 mask (contraction over
    partitions = features) -> [128 blocks, 256] = [s1 | s2] in PSUM,
    ACT copy -> bf16 st, PE flip both halves back to row-major PSUM
  - DVE+ACT: coefficient math on [128, 128] (reading stats straight from
    PSUM), a/b written as bf16
  - DVE apply: mul (bf16 2x mode) in place, add writes fp32 out tile
  - DMA out
"""

import sys

import numpy as np

for _p in ("/opt/trn_rl_repo",):
    if _p not in sys.path:
        sys.path.insert(0, _p)

import concourse.bacc as bacc
import concourse.bass as bass
import concourse.tile as tile
from concourse import mybir
from concourse.bass_utils import run_bass_kernel_spmd

F32 = mybir.dt.float32
BF16 = mybir.dt.bfloat16
ALU = mybir.AluOpType

N_CORES = 8
B_FULL = 4096          # total rows
N = 8192               # features
BLOCK = 16
NB = N // BLOCK        # 512 blocks
EPS = 1e-5
R = B_FULL // N_CORES  # 512 rows per core

CW = 2048              # column chunk width


def build_nc(rows: int = R, cols: int = N, cw: int = CW) -> bass.Bass:
    nb = cols // BLOCK
    nrt = rows // 128
    ncc = cols // cw
    nbw = cw // BLOCK   # blocks per chunk (128)
    spc = cw // 128     # 128-col sub-blocks per chunk (16)

    nc = bacc.Bacc("TRN2", target_bir_lowering=False, debug=False,
                   num_devices=N_CORES)
    x = nc.declare_dram_parameter("x", [rows, cols], F32, isOutput=False)
    scales = nc.declare_dram_parameter("scales", [nb], F32, isOutput=False)
    shifts = nc.declare_dram_parameter("shifts", [nb], F32, isOutput=False)
    ident = nc.declare_dram_parameter("ident", [128, 128], F32, isOutput=False)
    # maskall[f, k*128 + g] = 1 iff g == 8k + f//16: matmul k of a chunk
    # accumulates sub-block k's 8 block-sums into output partitions
    # 8k..8k+8 (PE out base partition must be 0 — masks route instead).
    mask = nc.declare_dram_parameter(
        "maskall", [128, spc * 128], F32, isOutput=False)
    out = nc.declare_dram_parameter("out", [rows, cols], F32, isOutput=True)

    with tile.TileContext(nc) as tc:
        with (
            tc.tile_pool(name="singles", bufs=1) as singles,
            tc.tile_pool(name="xp", bufs=3) as xp,
            tc.tile_pool(name="xbp", bufs=3) as xbp,
            tc.tile_pool(name="xsp", bufs=2) as xsp,
            tc.tile_pool(name="outp", bufs=3) as outp,
            tc.tile_pool(name="wsp", bufs=4) as wsp,
            tc.tile_pool(name="cofp", bufs=3) as cofp,
            tc.tile_pool(name="stp", bufs=2) as stp,
            tc.tile_pool(name="psA", bufs=2, space="PSUM") as psA,
            tc.tile_pool(name="psB", bufs=2, space="PSUM") as psB,
            tc.tile_pool(name="psF", bufs=2, space="PSUM") as psF,
        ):
            sc = singles.tile([128, nb], F32)
            sh = singles.tile([128, nb], F32)
            nc.gpsimd.dma_start(out=sc[:, :], in_=scales[:].partition_broadcast(128))
            nc.gpsimd.dma_start(out=sh[:, :], in_=shifts[:].partition_broadcast(128))
            eps_t = singles.tile([128, 1], F32)
            nc.vector.memset(eps_t[:, :], EPS)
            ident_f = singles.tile([128, 128], F32)
            mask_f = singles.tile([128, spc * 128], F32)
            nc.sync.dma_start(out=ident_f[:, :], in_=ident[:, :])
            nc.sync.dma_start(out=mask_f[:, :], in_=mask[:, :])
            # bf16 copies of the constants (exact: 0/1 values)
            ident_b = singles.tile([128, 128], BF16)
            mask_b = singles.tile([128, spc * 128], BF16)
            nc.scalar.copy(out=ident_b[:, :], in_=ident_f[:, :])
            nc.scalar.copy(out=mask_b[:, :], in_=mask_f[:, :])

            for rt in range(nrt):
                r0 = rt * 128
                for c in range(ncc):
                    sl = slice(c * cw, (c + 1) * cw)
                    gbsl = slice(c * nbw, (c + 1) * nbw)  # global block range
                    xt = xp.tile([128, cw], F32, tag="x")
                    nc.sync.dma_start(out=xt[:, :], in_=x[r0 : r0 + 128, sl])

                    # bf16 working copy
                    xb = xbp.tile([128, cw], BF16, tag="xb")
                    nc.scalar.copy(out=xb[:, :], in_=xt[:, :])

                    # transpose + interleave [xT_k | sqT_k] per 256 cols
                    xs = xsp.tile([128, 2 * cw], BF16, tag="xs")
                    xs3 = xs[:, :].rearrange("p (k n) -> p k n", n=256)
                    for half in range(cw // 1024):
                        xT = psA.tile([128, 1024], BF16, tag="xT")
                        for j in range(8):
                            col0 = half * 1024 + j * 128
                            nc.tensor.transpose(
                                xT[:, j * 128 : (j + 1) * 128],
                                xb[:, col0 : col0 + 128],
                                ident_b[:, :],
                            )
                        hs = slice(half * 8, (half + 1) * 8)
                        nc.scalar.copy(out=xs3[:, hs, 0:128], in_=xT[:, :])
                        nc.scalar.square(out=xs3[:, hs, 128:256], in_=xT[:, :])

                    # masked bf16 matmuls: [128 blocks, 256] = [s1 | s2]
                    scps = psB.tile([128, 256], F32, tag="s12")
                    for k in range(spc):
                        nc.tensor.matmul(
                            scps[:, :],
                            mask_b[:, k * 128 : (k + 1) * 128],
                            xs[:, k * 256 : (k + 1) * 256],
                            start=(k == 0), stop=(k == spc - 1),
                        )
                    st = stp.tile([128, 256], BF16, tag="st")
                    nc.scalar.copy(out=st[:, :], in_=scps[:, :])
                    # flip [block, row] -> [row, block]; both stats in one
                    # PSUM tile (bank budget)
                    fpB = psF.tile([128, 256], BF16, tag="fp")
                    nc.tensor.transpose(
                        fpB[:, 0:128], st[:, 0:128], ident_b[:, :])
                    nc.tensor.transpose(
                        fpB[:, 128:256], st[:, 128:256], ident_b[:, :])

                    # per-block a = scales/sqrt(var+eps), b = shifts - mean*a
                    # (all fp32 in SBUF; single bf16 convert of [a|b] at end)
                    ws = wsp.tile([128, 8 * nbw], F32, tag="ws")
                    mm = ws[:, 0 * nbw : 1 * nbw]
                    raw = ws[:, 1 * nbw : 2 * nbw]
                    sd = ws[:, 2 * nbw : 3 * nbw]
                    rstd = ws[:, 3 * nbw : 4 * nbw]
                    af = ws[:, 4 * nbw : 5 * nbw]
                    bf = ws[:, 5 * nbw : 6 * nbw]
                    s12f = ws[:, 6 * nbw : 8 * nbw]
                    s1f = ws[:, 6 * nbw : 7 * nbw]
                    s2f = ws[:, 7 * nbw : 8 * nbw]
                    cof = cofp.tile([128, 2 * nbw], BF16, tag="cof")
                    ab = cof[:, 0:nbw]
                    bb = cof[:, nbw : 2 * nbw]

                    nc.scalar.copy(out=s12f, in_=fpB[:, :])
                    nc.scalar.square(out=mm, in_=s1f)
                    nc.vector.scalar_tensor_tensor(
                        out=raw, in0=mm, scalar=-1.0 / BLOCK, in1=s2f,
                        op0=ALU.mult, op1=ALU.add,
                    )
                    nc.scalar.activation(
                        out=sd, in_=raw,
                        func=mybir.ActivationFunctionType.Sqrt,
                        bias=eps_t[:, :], scale=1.0 / (BLOCK - 1),
                    )
                    # rstd = 1/sd (2 DVE ops); mm reused as scratch
                    nc.vector.reciprocal_approx_accurate(
                        out=rstd, in_=sd, scratch=mm)
                    nc.vector.tensor_mul(out=af, in0=sc[:, gbsl], in1=rstd)
                    nc.vector.tensor_mul(out=raw, in0=s1f, in1=af)
                    nc.vector.scalar_tensor_tensor(
                        out=bf, in0=raw, scalar=-1.0 / BLOCK, in1=sh[:, gbsl],
                        op0=ALU.mult, op1=ALU.add,
                    )
                    # [a|b] -> bf16 in one op (DVE copy, 2 slots contiguous)
                    nc.vector.tensor_copy(cof[:, :], ws[:, 4 * nbw : 6 * nbw])

                    # apply: xb = xb * a (bf16 2x), out_f = xb + b (fp32 out)
                    xb3 = xb[:, :].rearrange("p (g b) -> p g b", b=BLOCK)
                    a3 = ab.unsqueeze(2).broadcast_to((128, nbw, BLOCK))
                    b3 = bb.unsqueeze(2).broadcast_to((128, nbw, BLOCK))
                    of = outp.tile([128, cw], F32, tag="of")
                    of3 = of[:, :].rearrange("p (g b) -> p g b", b=BLOCK)
                    nc.vector.tensor_mul(out=xb3, in0=xb3, in1=a3)
                    nc.vector.tensor_add(out=of3, in0=xb3, in1=b3)
                    nc.sync.dma_start(out=out[r0 : r0 + 128, sl],
                                      in_=of[:, :])
    nc.compile()
    return nc


def aux_inputs(cw: int = CW) -> dict:
    """Constant tensors fed alongside the real inputs."""
    spc = cw // 128
    maskall = np.zeros((128, spc * 128), np.float32)
    for k in range(spc):
        for f in range(128):
            maskall[f, k * 128 + 8 * k + f // BLOCK] = 1.0
    return {"ident": np.eye(128, dtype=np.float32), "maskall": maskall}


_NC_CACHE: dict = {}


def _get_nc() -> bass.Bass:
    if "nc" not in _NC_CACHE:
        _NC_CACHE["nc"] = build_nc()
    return _NC_CACHE["nc"]


def run_sharded(x, scales, shifts, trace: bool = False):
    """Run the SPMD kernel on 8 cores. Returns (out, BassKernelResults)."""
    x = np.ascontiguousarray(np.asarray(x, dtype=np.float32))
    scales = np.ascontiguousarray(np.asarray(scales, dtype=np.float32))
    shifts = np.ascontiguousarray(np.asarray(shifts, dtype=np.float32))
    assert x.shape == (B_FULL, N), x.shape
    nc = _get_nc()
    in_maps = [
        {"x": x[i * R : (i + 1) * R], "scales": scales, "shifts": shifts,
         **aux_inputs()}
        for i in range(N_CORES)
    ]
    res = run_bass_kernel_spmd(nc, in_maps, core_ids=list(range(N_CORES)), trace=trace)
    outs = [np.asarray(m["out"]) for m in res.results]
    return np.concatenate(outs, axis=0), res


def kernel(x, scales, shifts):
    out, _ = run_sharded(x, scales, shifts, trace=False)
    return out
